# revision 25
# baseline (speedup 1.0000x reference)
"""AFIA (attention + convs + layernorms) Trainium2 Bass kernel, 8 NeuronCores.

Problem: x (4, 128, 64, 64) f32 plus conv/LN params; output (4, 128, 64, 64).

Sharding: data-parallel over batch (4) x query-row-halves (2) = 8 cores, no
collectives. Each core holds the full image of its batch (needed for K/V and
dense attention over all 4096 tokens) and computes one 32-row half of the
output. All cores run the same graph; per-core differences are entirely in
the input data (full image, halo slice, edge-mask flags).

On-device layout: channels C=128 on SBUF partitions, tokens along free dim.
 - LayerNorm over channels via TensorE ones-matmul moments (mean/mean-square
   broadcast across partitions by the matmul itself), rstd = exp(-0.5*ln(var+eps))
   on ScalarE (keeps exp+ln in one ACT table set), apply on VectorE.
   ln1 gamma is folded into consumer conv weights host-side (beta folds into
   conv biases; exact for the 1x1 convs, and for the 3x3 path beta==0).
 - 1x1 convs: single matmul per 512-token chunk.
 - 3x3 convs / deconvs: 9 shift-matmuls accumulating in PSUM, reading from
   zero-padded (rows+2, 66) SBUF images.
 - Attention without max-subtraction (scores/sqrt(C) stay within ~+-6):
   S^T tiles = K-block^T @ Q-chunk; E = exp on ScalarE; O^T accumulated with
   lhsT=E-slices, rhs=[V^T | ones] so column 128 accumulates the softmax
   denominator; drain = relu(O * 1/den) fused on ScalarE; PE-transpose back.
 - bf16 matmul inputs everywhere, f32 PSUM accumulation (validated 4e-3 rel).
"""
import math
import os
import sys
import types
from contextlib import ExitStack

import numpy as np
import ml_dtypes

sys.path.insert(0, "/opt/trn_rl_repo")

import concourse.bass as bass
import concourse.tile as tile
from concourse.masks import make_identity
from concourse import bacc, mybir
from concourse.bass_utils import run_bass_kernel_spmd

F32 = mybir.dt.float32
BF16 = mybir.dt.bfloat16
AF = mybir.ActivationFunctionType

C = 128
H = W = 64
HW = H * W
NCORES = 8
NQ = 32 * W          # 2048 query tokens per core
EPS = 1e-5
SCALE = 1.0 / math.sqrt(float(C))


def _bf16(a):
    return np.ascontiguousarray(np.asarray(a, np.float32).astype(ml_dtypes.bfloat16))


def _f32(a):
    return np.ascontiguousarray(np.asarray(a, np.float32))


# --------------------------------------------------------------------------
# host-side input prep
# --------------------------------------------------------------------------

def _prep_weights(params):
    p = {k: np.asarray(v, np.float32) for k, v in params.items()}
    g1, b1 = p["ln1_g"], p["ln1_b"]
    out = {}

    def conv1x1_lhsT(w, bias):
        # w (O,I,1,1): lhsT (i,o) with ln1 gamma folded on input channels,
        # beta folded into bias (exact for 1x1).
        w2 = w[:, :, 0, 0]                         # (o, i)
        lhsT = (w2 * g1[None, :]).T                # (i, o)
        beff = bias + w2 @ b1
        return _bf16(lhsT), _f32(beff.reshape(C, 1))

    out["wq1T"], out["bq1e"] = conv1x1_lhsT(p["wq1"], p["bq1"])
    out["wk1T"], out["bk1e"] = conv1x1_lhsT(p["wk1"], p["bk1"])
    out["wv1T"], out["bv1e"] = conv1x1_lhsT(p["wv1"], p["bv1"])
    out["woT"] = _bf16(p["wo"][:, :, 0, 0].T)      # no LN in front of wo

    def deconv_lhsT(w):
        # torch ConvTranspose2d weight (I,O,3,3); equivalent regular conv
        # weight w_conv[o,i,dy,dx] = w[i,o,2-dy,2-dx]; lhsT[i,o] per (dy,dx).
        # store as (ci, 9, co) for a partition-major SBUF tile.
        lhsT = np.empty((C, 9, C), np.float32)
        for dy in range(3):
            for dx in range(3):
                lhsT[:, dy * 3 + dx, :] = w[:, :, 2 - dy, 2 - dx]
        return _bf16(lhsT)

    out["wqdT"] = deconv_lhsT(p["wqd"])
    out["wkdT"] = deconv_lhsT(p["wkd"])
    out["wvdT"] = deconv_lhsT(p["wvd"])

    # ws1: channel shuffle (groups=4) folded into input channels, plus ln1
    # gamma fold; beta fold into bias (beta==0 in this problem; the 3x3
    # border contribution of a nonzero beta is not position-corrected).
    groups = 4
    perm = np.empty(C, np.int64)
    for g in range(groups):
        for i2 in range(C // groups):
            perm[i2 * groups + g] = g * (C // groups) + i2
    ws1 = p["ws1"]
    ws1_eff = np.zeros_like(ws1)
    ws1_eff[:, perm, :, :] = ws1                   # (o, i, dy, dx), i = xn channel

    def conv3x3_lhsT(w, gamma=None):
        lhsT = np.empty((C, 9, C), np.float32)
        for dy in range(3):
            for dx in range(3):
                ww = w[:, :, dy, dx]               # (o, i)
                if gamma is not None:
                    ww = ww * gamma[None, :]
                lhsT[:, dy * 3 + dx, :] = ww.T
        return lhsT

    out["ws1T"] = _bf16(conv3x3_lhsT(ws1_eff, g1))
    bs1_eff = p["bs1"] + ws1_eff.sum(axis=(2, 3)) @ b1
    out["bs1e"] = _f32(bs1_eff.reshape(C, 1))
    out["ws2T"] = _bf16(conv3x3_lhsT(p["ws2"]))
    out["bs2bo"] = _f32((p["bs2"] + p["bo"]).reshape(C, 1))

    for nm in ("bqd", "bkd", "bvd"):
        out[nm] = _f32(p[nm].reshape(C, 1))
    out["ln2_g"] = _f32(p["ln2_g"].reshape(C, 1))
    out["ln2_b"] = _f32(p["ln2_b"].reshape(C, 1))
    return out


def _prep_core_inputs(x, wp):
    """Per-core input dicts. core = 2*b + h; r0 = 32*h."""
    maps = []
    for core in range(NCORES):
        b, h = core // 2, core % 2
        r0 = 32 * h
        xh = np.zeros((C, 36, W), np.float32)
        lo, hi = r0 - 2, r0 + 34
        slo, shi = max(lo, 0), min(hi, H)
        xh[:, slo - lo:shi - lo, :] = x[b, :, slo:shi, :]
        m = {
            "xh": _f32(xh.reshape(C, 36 * W)),
            "flag_top": np.full((C, 1), 0.0 if r0 == 0 else 1.0, np.float32),
            "flag_bot": np.full((C, 1), 0.0 if r0 + 32 == H else 1.0, np.float32),
        }
        m.update(wp)
        maps.append(m)
    return maps


# --------------------------------------------------------------------------
# graph builder
# --------------------------------------------------------------------------

def _build():
    nc = bacc.Bacc("TRN2", target_bir_lowering=False, debug=False,
                   num_devices=NCORES)

    def din(name, shape, dt):
        return nc.dram_tensor(name, list(shape), dt, kind="ExternalInput").ap()

    xh_d = din("xh", (C, 36 * W), F32)
    ft_d = din("flag_top", (C, 1), F32)
    fb_d = din("flag_bot", (C, 1), F32)
    w1_d = {nm: din(nm, (C, C), BF16) for nm in ("wq1T", "wk1T", "wv1T", "woT")}
    wd_d = {nm: din(nm, (C, 9, C), BF16)
            for nm in ("wqdT", "wkdT", "wvdT", "ws1T", "ws2T")}
    bias_d = {nm: din(nm, (C, 1), F32)
              for nm in ("bq1e", "bk1e", "bv1e", "bqd", "bkd", "bvd",
                         "bs1e", "bs2bo", "ln2_g", "ln2_b")}
    out_d = nc.dram_tensor("out", [C, NQ], F32, kind="ExternalOutput").ap()

    with tile.TileContext(nc) as tc:
        with ExitStack() as ctx:
            _emit(ctx, tc, nc, xh_d, ft_d, fb_d, w1_d, wd_d, bias_d, out_d)
    nc.compile()
    return nc


def _emit(ctx, tc, nc, xh_d, ft_d, fb_d, w1_d, wd_d, bias_d, out_d):
    sg = ctx.enter_context(tc.tile_pool(name="singles", bufs=1))

    # ---- load inputs (xf/xh stream in per 512-token chunk so LN starts
    #      after the first 256KB instead of the full 2MB) ----
    xh = sg.tile([C, 36 * W], F32, name="xh_sb")
    for c in range(5):
        lo, hi = 512 * c, min(512 * (c + 1), 36 * W)
        nc.sync.dma_start(xh[:, lo:hi], xh_d[:, lo:hi])
    flag_top = sg.tile([C, 1], F32, name="ft_sb")
    nc.sync.dma_start(flag_top[:], ft_d)
    flag_bot = sg.tile([C, 1], F32, name="fb_sb")
    nc.sync.dma_start(flag_bot[:], fb_d)
    w1 = {}
    for nm, d in w1_d.items():
        w1[nm] = sg.tile([C, C], BF16, name=nm + "_sb")
        nc.sync.dma_start(w1[nm][:], d)
    wd = {}
    for nm, d in wd_d.items():
        wd[nm] = sg.tile([C, 9, C], BF16, name=nm + "_sb")
        nc.sync.dma_start(wd[nm][:], d)
    bias = {}
    for nm, d in bias_d.items():
        bias[nm] = sg.tile([C, 1], F32, name=nm + "_sb")
        nc.sync.dma_start(bias[nm][:], d)

    onesC = sg.tile([C, C], BF16, name="onesC")
    nc.gpsimd.memset(onesC[:], 1.0 / C)
    ident = sg.tile([C, C], BF16, name="ident")
    make_identity(nc, ident[:])
    onesF = sg.tile([C, C], BF16, name="onesF")
    nc.gpsimd.memset(onesF[:], 1.0)
    zero_t = sg.tile([C, 1], F32, name="zero_t")
    nc.vector.memset(zero_t[:], 0.0)
    eps_t = sg.tile([C, 1], F32, name="eps_t")
    nc.vector.memset(eps_t[:], EPS)

    # ---- big activation tiles ----
    # K and V are computed as 32-row halves from the same halo as Q (the
    # three chains are structurally identical), then the full K/V are
    # assembled with a pair-wise AllGather (cores 2b and 2b+1 hold the two
    # halves of batch b).
    xnh_pad = sg.tile([C, 36, 66], BF16, name="xnh_pad")  # LN1(halo), padded
    k1p = sg.tile([C, 36, 66], BF16, name="k1p")
    v1p = sg.tile([C, 36, 66], BF16, name="v1p")
    q1p = sg.tile([C, 36, 66], BF16, name="q1p")
    s1p = sg.tile([C, 36, 66], BF16, name="s1p")
    KVh = sg.tile([C, HW], BF16, name="KVh")   # [K-half | V-half]
    K_cn = sg.tile([C, HW], BF16, name="K_cn")
    V_cn = sg.tile([C, HW], BF16, name="V_cn")
    Q_cn = sg.tile([C, NQ], BF16, name="Q_cn")
    VT_STRIDE = 160   # 320B row stride keeps each slice 32B-aligned for xbar DMA
    Vt_aug = sg.tile([C, 32, VT_STRIDE], BF16, name="Vt_aug")
    O_cn = sg.tile([C, NQ], BF16, name="O_cn")
    sum_sb = sg.tile([C, NQ], F32, name="sum_sb")
    out_sb = sg.tile([C, NQ], F32, name="out_sb")

    dramp = ctx.enter_context(tc.tile_pool(name="dramp", bufs=1, space="DRAM"))
    kv_in = dramp.tile([C, HW], BF16, name="kv_in")
    kv_out = dramp.tile([2 * C, HW], BF16, name="kv_out")

    # zero only the pad borders (full-tile memsets on the gpsimd queue were
    # delaying the LN pipeline); interiors are fully written by conv drains
    for t in (xnh_pad, k1p, v1p, q1p, s1p):
        rows = t.shape[1]
        nc.vector.memset(t[:, 0, :], 0.0)
        nc.vector.memset(t[:, rows - 1, :], 0.0)
        nc.gpsimd.memset(t[:, :, 0], 0.0)
        nc.gpsimd.memset(t[:, :, 65], 0.0)
    nc.vector.memset(Vt_aug[:, :, C:C + 1], 1.0)

    # ---- phase A pools ----
    phA = ExitStack()
    psLN = phA.enter_context(tc.tile_pool(name="psLN", bufs=2, space="PSUM"))
    psCV = phA.enter_context(tc.tile_pool(name="psCV", bufs=3, space="PSUM"))
    psW = phA.enter_context(tc.tile_pool(name="psW", bufs=1, space="PSUM"))
    lnp = phA.enter_context(tc.tile_pool(name="lnp", bufs=3))

    # PE warm-up: ~5us of back-to-back matmuls at t~0 trips the HAM clock
    # gate (cold PE runs at 1.2 GHz; the LN phase alone is too sparse to
    # warm it before the conv burst arrives).
    warm_ps = psW.tile([C, C], F32, name="warm_ps")
    for _ in range(24):
        nc.tensor.matmul(warm_ps[:], lhsT=onesC[:], rhs=onesC[:],
                         start=True, stop=True)

    def ln_chunk(src_ap, dst_ap, F):
        """LayerNorm-over-channels for one chunk of F tokens (no gamma/beta;
        gamma/beta folded into consumer conv weights host-side).
        src: (C,F) f32 SBUF ap; dst: (C,F)-shaped bf16 ap (possibly strided).
        rstd = 1/sqrt(var+eps): Sqrt on ACT (one table set), recip on DVE.
        mu^2 and var on GpSimd (frees DVE); mean-sub fused as (xb-mu)*R."""
        xb = lnp.tile([C, 512], BF16, name="ln_xb")
        nc.scalar.activation(xb[:, :F], src_ap, AF.Copy)
        sq = lnp.tile([C, 512], BF16, name="ln_sq")
        nc.vector.tensor_mul(sq[:, :F], xb[:, :F], xb[:, :F])
        mu_p = psLN.tile([C, 512], F32, name="ln_mup")
        nc.tensor.matmul(mu_p[:, :F], lhsT=onesC[:], rhs=xb[:, :F],
                         start=True, stop=True)
        msq_p = psLN.tile([C, 512], F32, name="ln_msqp")
        nc.tensor.matmul(msq_p[:, :F], lhsT=onesC[:], rhs=sq[:, :F],
                         start=True, stop=True)
        mu = lnp.tile([C, 512], F32, name="ln_mu")
        nc.scalar.activation(mu[:, :F], mu_p[:, :F], AF.Copy)
        msq = lnp.tile([C, 512], F32, name="ln_msq")
        nc.scalar.activation(msq[:, :F], msq_p[:, :F], AF.Copy)
        t2 = lnp.tile([C, 512], F32, name="ln_t2")
        nc.gpsimd.tensor_mul(t2[:, :F], mu[:, :F], mu[:, :F])
        var = lnp.tile([C, 512], F32, name="ln_var")
        nc.gpsimd.tensor_sub(var[:, :F], msq[:, :F], t2[:, :F])
        sd = lnp.tile([C, 512], F32, name="ln_sd")
        nc.scalar.activation(sd[:, :F], var[:, :F], AF.Sqrt, bias=eps_t[:])
        R = lnp.tile([C, 512], F32, name="ln_R")
        nc.vector.reciprocal_approx_fast(R[:, :F], sd[:, :F])
        xc = lnp.tile([C, 512], BF16, name="ln_xc")
        nc.vector.tensor_sub(xc[:, :F], xb[:, :F], mu_p[:, :F])
        nc.vector.tensor_mul(dst_ap, xc[:, :F], R[:, :F])

    # (LN chunks are interleaved with the conv chains below to keep the
    #  TensorEngine fed -- a cold PE runs at 1.2 GHz until ~3.4us of
    #  sustained activity.)
    hgroups = [(0, 8), (8, 8), (16, 8), (24, 8), (32, 4)]

    def conv1x1(lhsT, rhs_ap, dst_ap, F, bias_ap, func=AF.Identity):
        ps = psCV.tile([C, 512], F32, name="cv_ps")
        nc.tensor.matmul(ps[:, :F], lhsT=lhsT, rhs=rhs_ap, start=True, stop=True)
        nc.scalar.activation(dst_ap, ps[:, :F], func, bias=bias_ap)

    def conv3x3(wtile, src_pad, row0, nr, dst_ap, bias_ap, func=AF.Identity):
        """out rows use src_pad rows row0+dy .. row0+dy+nr, cols dx..dx+64."""
        F = nr * W
        ps = psCV.tile([C, 512], F32, name="cv_ps")
        for s in range(9):
            dy, dx = s // 3, s % 3
            nc.tensor.matmul(
                ps[:, :F],
                lhsT=wtile[:, s, :],
                rhs=src_pad[:, row0 + dy:row0 + dy + nr, dx:dx + W],
                start=(s == 0), stop=(s == 8))
        nc.scalar.activation(dst_ap, ps[:, :F], func, bias=bias_ap)

    # ---- LN1 on the halo, interleaved with the q1/k1/v1 1x1 convs
    #      (group hi reads xnh rows 8hi+1..8hi+8 = LN chunks hi and hi+1,
    #       so the convs trail the LN chunks by one) ----
    qgroups = [(1, 8), (9, 8), (17, 8), (25, 8), (33, 2)]

    def c1group(hi):
        qi0, qnr = qgroups[hi]
        for wname, bname, dstp in (("wk1T", "bk1e", k1p), ("wv1T", "bv1e", v1p),
                                   ("wq1T", "bq1e", q1p)):
            conv1x1(w1[wname][:], xnh_pad[:, qi0:qi0 + qnr, 1:65],
                    dstp[:, qi0:qi0 + qnr, 1:65], qnr * W, bias[bname][:])

    def kvd_group(r):
        # K/V half deconv row-group r, streamed straight out to the
        # collective input buffer
        conv3x3(wd["wkdT"], k1p, 8 * r + 1, 8,
                KVh[:, 512 * r:512 * (r + 1)], bias["bkd"][:])
        nc.sync.dma_start(kv_in[:, 512 * r:512 * (r + 1)],
                          KVh[:, 512 * r:512 * (r + 1)])
        conv3x3(wd["wvdT"], v1p, 8 * r + 1, 8,
                KVh[:, NQ + 512 * r:NQ + 512 * (r + 1)], bias["bvd"][:])
        nc.sync.dma_start(kv_in[:, NQ + 512 * r:NQ + 512 * (r + 1)],
                          KVh[:, NQ + 512 * r:NQ + 512 * (r + 1)])

    # LN chunks -> 1x1 convs (trailing 1) -> K/V deconv groups (trailing 2),
    # so the AllGather can issue as early as possible.
    for hi, (i0, nr) in enumerate(hgroups):
        F = nr * W
        ln_chunk(xh[:, i0 * W:i0 * W + F],
                 xnh_pad[:, i0:i0 + nr, 1:65], F)
        if hi >= 1:
            c1group(hi - 1)
        if hi == 2:
            for t in (k1p, v1p, q1p):
                nc.vector.tensor_scalar_mul(t[:, 1, 1:65], t[:, 1, 1:65],
                                            flag_top[:])
        if hi >= 2:
            kvd_group(hi - 2)
    c1group(4)
    for t in (k1p, v1p, q1p):
        nc.vector.tensor_scalar_mul(t[:, 34, 1:65], t[:, 34, 1:65], flag_bot[:])
    kvd_group(3)

    nc.gpsimd.collective_compute(
        "AllGather", mybir.AluOpType.bypass,
        ins=[kv_in[:]], outs=[kv_out[:]],
        replica_groups=[[0, 1], [2, 3], [4, 5], [6, 7]])

    # ---- work that overlaps the collective: Q deconv, sc chain ----
    for r in range(4):
        conv3x3(wd["wqdT"], q1p, 8 * r + 1, 8,
                Q_cn[:, 512 * r:512 * (r + 1)], bias["bqd"][:])

    sgroups = [(0, 8), (8, 8), (16, 8), (24, 8), (32, 2)]   # 34 s1 rows
    for (i0, nr) in sgroups:   # s1 rows i0..i0+nr-1 (abs r0-1+i0..)
        conv3x3(wd["ws1T"], xnh_pad, i0, nr,
                s1p[:, 1 + i0:1 + i0 + nr, 1:65], bias["bs1e"][:], func=AF.Relu)
    nc.vector.tensor_scalar_mul(s1p[:, 1, 1:65], s1p[:, 1, 1:65], flag_top[:])
    nc.vector.tensor_scalar_mul(s1p[:, 34, 1:65], s1p[:, 34, 1:65], flag_bot[:])
    for r in range(4):
        conv3x3(wd["ws2T"], s1p, 8 * r + 1, 8,
                sum_sb[:, 512 * r:512 * (r + 1)], bias["bs2bo"][:])
    # add residual (raw x rows r0..r0+31 live in xh rows 2..33)
    for r in range(4):
        nc.vector.tensor_add(sum_sb[:, 512 * r:512 * (r + 1)],
                             sum_sb[:, 512 * r:512 * (r + 1)],
                             xh[:, 2 * W + 512 * r:2 * W + 512 * (r + 1)])

    # ---- assemble full K/V from the gathered halves; V transposed on the
    #      TensorEngine (the xbar DMA path serialized ~39us on one queue) ----
    for half in range(2):
        nc.sync.dma_start(K_cn[:, NQ * half:NQ * (half + 1)],
                          kv_out[C * half:C * (half + 1), 0:NQ])
        nc.sync.dma_start(V_cn[:, NQ * half:NQ * (half + 1)],
                          kv_out[C * half:C * (half + 1), NQ:HW])

    phA.close()

    def ln2_chunk(src_ap, dst_ap, F):
        # final LayerNorm chunk: f32 apply with ln2 gamma/beta
        xb = lnp2.tile([C, 512], BF16, name="l2_xb")
        nc.vector.tensor_copy(xb[:, :F], src_ap)
        sq = lnp2.tile([C, 512], BF16, name="l2_sq")
        nc.vector.tensor_mul(sq[:, :F], xb[:, :F], xb[:, :F])
        mu_p = psL2.tile([C, 512], F32, name="l2_mup")
        nc.tensor.matmul(mu_p[:, :F], lhsT=onesC[:], rhs=xb[:, :F],
                         start=True, stop=True)
        msq_p = psL2.tile([C, 512], F32, name="l2_msqp")
        nc.tensor.matmul(msq_p[:, :F], lhsT=onesC[:], rhs=sq[:, :F],
                         start=True, stop=True)
        mu = lnp2.tile([C, 512], F32, name="l2_mu")
        nc.vector.tensor_copy(mu[:, :F], mu_p[:, :F])
        t2 = lnp2.tile([C, 512], F32, name="l2_t2")
        nc.vector.tensor_mul(t2[:, :F], mu[:, :F], mu[:, :F])
        var = lnp2.tile([C, 512], F32, name="l2_var")
        nc.vector.tensor_sub(var[:, :F], msq_p[:, :F], t2[:, :F])
        sd = lnp2.tile([C, 512], F32, name="l2_sd")
        nc.scalar.activation(sd[:, :F], var[:, :F], AF.Sqrt, bias=eps_t[:])
        R = lnp2.tile([C, 512], F32, name="l2_R")
        nc.vector.reciprocal_approx_fast(R[:, :F], sd[:, :F])
        xc = lnp2.tile([C, 512], F32, name="l2_xc")
        nc.vector.tensor_sub(xc[:, :F], src_ap, mu[:, :F])
        xcr = lnp2.tile([C, 512], F32, name="l2_xcr")
        nc.vector.tensor_mul(xcr[:, :F], xc[:, :F], R[:, :F])
        nc.vector.tensor_scalar(
            out=dst_ap, in0=xcr[:, :F],
            scalar1=bias["ln2_g"][:], scalar2=bias["ln2_b"][:],
            op0=mybir.AluOpType.mult, op1=mybir.AluOpType.add)

    # ---- attention ----
    phB = ExitStack()
    psS = phB.enter_context(tc.tile_pool(name="psS", bufs=2, space="PSUM"))
    psO = phB.enter_context(tc.tile_pool(name="psO", bufs=2, space="PSUM"))
    psC = phB.enter_context(tc.tile_pool(name="psC", bufs=1, space="PSUM"))
    psL2 = phB.enter_context(tc.tile_pool(name="psL2", bufs=1, space="PSUM"))
    sbE = phB.enter_context(tc.tile_pool(name="sbE", bufs=4))
    sbOT = phB.enter_context(tc.tile_pool(name="sbOT", bufs=2))
    lnp2 = phB.enter_context(tc.tile_pool(name="lnp2", bufs=2))

    for m in range(32):
        tp = psS.tile([C, C], BF16, name="vt_ps", tag="s_ps")
        nc.tensor.transpose(tp[:], V_cn[:, C * m:C * (m + 1)], ident[:])
        nc.vector.tensor_copy(Vt_aug[:, m, 0:C], tp[:])

    for g in range(4):
        qs = Q_cn[:, 512 * g:512 * (g + 1)]
        o_ps = psO.tile([C, 512], F32, name="o_ps")
        den_ps = psO.tile([C, 512], F32, name="den_ps", bufs=1)
        for m in range(32):
            s_ps = psS.tile([C, 512], F32, name="s_ps")
            nc.tensor.matmul(s_ps[:], lhsT=K_cn[:, C * m:C * (m + 1)],
                             rhs=qs, start=True, stop=True)
            e = sbE.tile([C, 512], BF16, name="e_t")
            nc.scalar.activation(e[:], s_ps[:], AF.Exp, scale=SCALE, bias=zero_t[:])
            nc.tensor.matmul(o_ps[:], lhsT=Vt_aug[:, m, 0:C], rhs=e[:],
                             start=(m == 0), stop=(m == 31))
            nc.tensor.matmul(den_ps[:], lhsT=onesF[:], rhs=e[:],
                             start=(m == 0), stop=(m == 31))
        ro = sbOT.tile([C, 512], BF16, name="ro")
        nc.scalar.activation(ro[:], o_ps[:], AF.Relu, bias=zero_t[:])
        rden = sbOT.tile([C, 512], F32, name="rden")
        nc.vector.reciprocal_approx_fast(rden[:], den_ps[:])
        nc.vector.tensor_mul(O_cn[:, 512 * g:512 * (g + 1)], ro[:], rden[:])
        o2 = psC.tile([C, 512], F32, name="wo_ps")
        nc.tensor.matmul(o2[:], lhsT=w1["woT"][:],
                         rhs=O_cn[:, 512 * g:512 * (g + 1)], start=True, stop=True)
        nc.vector.tensor_add(sum_sb[:, 512 * g:512 * (g + 1)],
                             sum_sb[:, 512 * g:512 * (g + 1)], o2[:])
        ln2_chunk(sum_sb[:, 512 * g:512 * (g + 1)],
                  out_sb[:, 512 * g:512 * (g + 1)], 512)
    phB.close()

    nc.sync.dma_start(out_d, out_sb[:])


# --------------------------------------------------------------------------
# public entry point
# --------------------------------------------------------------------------

_CACHED = {}


def _get_nc():
    if "nc" not in _CACHED:
        _CACHED["nc"] = _build()
    return _CACHED["nc"]


def kernel(x, params):
    x = np.asarray(x, np.float32)
    wp = _prep_weights(params)
    in_maps = _prep_core_inputs(x, wp)
    nc = _get_nc()
    res = run_bass_kernel_spmd(nc, in_maps, list(range(NCORES)))
    out = np.empty((4, C, H, W), np.float32)
    for core in range(NCORES):
        b, h = core // 2, core % 2
        r0 = 32 * h
        out[b, :, r0:r0 + 32, :] = res.results[core]["out"].reshape(C, 32, W)
    return out


# revision 28
# speedup vs baseline: 1.2206x; 1.2206x over previous
"""AFIA (attention + convs + layernorms) Trainium2 Bass kernel, 8 NeuronCores.

Problem: x (4, 128, 64, 64) f32 plus conv/LN params; output (4, 128, 64, 64).

Sharding: data-parallel over batch (4) x query-row-halves (2) = 8 cores, no
collectives. Each core holds the full image of its batch (needed for K/V and
dense attention over all 4096 tokens) and computes one 32-row half of the
output. All cores run the same graph; per-core differences are entirely in
the input data (full image, halo slice, edge-mask flags).

On-device layout: channels C=128 on SBUF partitions, tokens along free dim.
 - LayerNorm over channels via TensorE ones-matmul moments (mean/mean-square
   broadcast across partitions by the matmul itself), rstd = exp(-0.5*ln(var+eps))
   on ScalarE (keeps exp+ln in one ACT table set), apply on VectorE.
   ln1 gamma is folded into consumer conv weights host-side (beta folds into
   conv biases; exact for the 1x1 convs, and for the 3x3 path beta==0).
 - 1x1 convs: single matmul per 512-token chunk.
 - 3x3 convs / deconvs: 9 shift-matmuls accumulating in PSUM, reading from
   zero-padded (rows+2, 66) SBUF images.
 - Attention without max-subtraction (scores/sqrt(C) stay within ~+-6):
   S^T tiles = K-block^T @ Q-chunk; E = exp on ScalarE; O^T accumulated with
   lhsT=E-slices, rhs=[V^T | ones] so column 128 accumulates the softmax
   denominator; drain = relu(O * 1/den) fused on ScalarE; PE-transpose back.
 - bf16 matmul inputs everywhere, f32 PSUM accumulation (validated 4e-3 rel).
"""
import math
import os
import sys
import types
from contextlib import ExitStack

import numpy as np
import ml_dtypes

sys.path.insert(0, "/opt/trn_rl_repo")

import concourse.bass as bass
import concourse.tile as tile
from concourse.masks import make_identity
from concourse import bacc, mybir
from concourse.bass_utils import run_bass_kernel_spmd

F32 = mybir.dt.float32
BF16 = mybir.dt.bfloat16
AF = mybir.ActivationFunctionType

C = 128
H = W = 64
HW = H * W
NCORES = 8
NQ = 32 * W          # 2048 query tokens per core
EPS = 1e-5
SCALE = 1.0 / math.sqrt(float(C))


def _bf16(a):
    return np.ascontiguousarray(np.asarray(a, np.float32).astype(ml_dtypes.bfloat16))


def _f32(a):
    return np.ascontiguousarray(np.asarray(a, np.float32))


# --------------------------------------------------------------------------
# host-side input prep
# --------------------------------------------------------------------------

def _prep_weights(params):
    p = {k: np.asarray(v, np.float32) for k, v in params.items()}
    g1, b1 = p["ln1_g"], p["ln1_b"]
    out = {}

    def conv1x1_lhsT(w, bias):
        # w (O,I,1,1): lhsT (i,o) with ln1 gamma folded on input channels,
        # beta folded into bias (exact for 1x1).
        w2 = w[:, :, 0, 0]                         # (o, i)
        lhsT = (w2 * g1[None, :]).T                # (i, o)
        beff = bias + w2 @ b1
        return _bf16(lhsT), _f32(beff.reshape(C, 1))

    out["wq1T"], out["bq1e"] = conv1x1_lhsT(p["wq1"], p["bq1"])
    out["wk1T"], out["bk1e"] = conv1x1_lhsT(p["wk1"], p["bk1"])
    out["wv1T"], out["bv1e"] = conv1x1_lhsT(p["wv1"], p["bv1"])
    out["woT"] = _bf16(p["wo"][:, :, 0, 0].T)      # no LN in front of wo

    def deconv_lhsT(w):
        # torch ConvTranspose2d weight (I,O,3,3); equivalent regular conv
        # weight w_conv[o,i,dy,dx] = w[i,o,2-dy,2-dx]; lhsT[i,o] per (dy,dx).
        # store as (ci, 9, co) for a partition-major SBUF tile.
        lhsT = np.empty((C, 9, C), np.float32)
        for dy in range(3):
            for dx in range(3):
                lhsT[:, dy * 3 + dx, :] = w[:, :, 2 - dy, 2 - dx]
        return _bf16(lhsT)

    out["wqdT"] = deconv_lhsT(p["wqd"])
    out["wkdT"] = deconv_lhsT(p["wkd"])
    out["wvdT"] = deconv_lhsT(p["wvd"])

    # ws1: channel shuffle (groups=4) folded into input channels, plus ln1
    # gamma fold; beta fold into bias (beta==0 in this problem; the 3x3
    # border contribution of a nonzero beta is not position-corrected).
    groups = 4
    perm = np.empty(C, np.int64)
    for g in range(groups):
        for i2 in range(C // groups):
            perm[i2 * groups + g] = g * (C // groups) + i2
    ws1 = p["ws1"]
    ws1_eff = np.zeros_like(ws1)
    ws1_eff[:, perm, :, :] = ws1                   # (o, i, dy, dx), i = xn channel

    def conv3x3_lhsT(w, gamma=None):
        lhsT = np.empty((C, 9, C), np.float32)
        for dy in range(3):
            for dx in range(3):
                ww = w[:, :, dy, dx]               # (o, i)
                if gamma is not None:
                    ww = ww * gamma[None, :]
                lhsT[:, dy * 3 + dx, :] = ww.T
        return lhsT

    out["ws1T"] = _bf16(conv3x3_lhsT(ws1_eff, g1))
    bs1_eff = p["bs1"] + ws1_eff.sum(axis=(2, 3)) @ b1
    out["bs1e"] = _f32(bs1_eff.reshape(C, 1))
    out["ws2T"] = _bf16(conv3x3_lhsT(p["ws2"]))
    out["bs2bo"] = _f32((p["bs2"] + p["bo"]).reshape(C, 1))

    for nm in ("bqd", "bkd", "bvd"):
        out[nm] = _f32(p[nm].reshape(C, 1))
    out["ln2_g"] = _f32(p["ln2_g"].reshape(C, 1))
    out["ln2_b"] = _f32(p["ln2_b"].reshape(C, 1))
    return out


def _prep_core_inputs(x, wp):
    """Per-core input dicts. core = 2*b + h; r0 = 32*h."""
    maps = []
    for core in range(NCORES):
        b, h = core // 2, core % 2
        r0 = 32 * h
        xh = np.zeros((C, 36, W), np.float32)
        lo, hi = r0 - 2, r0 + 34
        slo, shi = max(lo, 0), min(hi, H)
        xh[:, slo - lo:shi - lo, :] = x[b, :, slo:shi, :]
        m = {
            "xh": _f32(xh.reshape(C, 36 * W)),
            "flag_top": np.full((C, 1), 0.0 if r0 == 0 else 1.0, np.float32),
            "flag_bot": np.full((C, 1), 0.0 if r0 + 32 == H else 1.0, np.float32),
        }
        m.update(wp)
        maps.append(m)
    return maps


# --------------------------------------------------------------------------
# graph builder
# --------------------------------------------------------------------------

def _build():
    nc = bacc.Bacc("TRN2", target_bir_lowering=False, debug=False,
                   num_devices=NCORES)

    def din(name, shape, dt):
        return nc.dram_tensor(name, list(shape), dt, kind="ExternalInput").ap()

    xh_d = din("xh", (C, 36 * W), F32)
    ft_d = din("flag_top", (C, 1), F32)
    fb_d = din("flag_bot", (C, 1), F32)
    w1_d = {nm: din(nm, (C, C), BF16) for nm in ("wq1T", "wk1T", "wv1T", "woT")}
    wd_d = {nm: din(nm, (C, 9, C), BF16)
            for nm in ("wqdT", "wkdT", "wvdT", "ws1T", "ws2T")}
    bias_d = {nm: din(nm, (C, 1), F32)
              for nm in ("bq1e", "bk1e", "bv1e", "bqd", "bkd", "bvd",
                         "bs1e", "bs2bo", "ln2_g", "ln2_b")}
    out_d = nc.dram_tensor("out", [C, NQ], F32, kind="ExternalOutput").ap()

    with tile.TileContext(nc) as tc:
        with ExitStack() as ctx:
            _emit(ctx, tc, nc, xh_d, ft_d, fb_d, w1_d, wd_d, bias_d, out_d)
    nc.compile()
    return nc


def _emit(ctx, tc, nc, xh_d, ft_d, fb_d, w1_d, wd_d, bias_d, out_d):
    sg = ctx.enter_context(tc.tile_pool(name="singles", bufs=1))

    # ---- load inputs (xf/xh stream in per 512-token chunk so LN starts
    #      after the first 256KB instead of the full 2MB) ----
    xh = sg.tile([C, 36 * W], F32, name="xh_sb")
    for c in range(5):
        lo, hi = 512 * c, min(512 * (c + 1), 36 * W)
        nc.sync.dma_start(xh[:, lo:hi], xh_d[:, lo:hi])
    flag_top = sg.tile([C, 1], F32, name="ft_sb")
    nc.sync.dma_start(flag_top[:], ft_d)
    flag_bot = sg.tile([C, 1], F32, name="fb_sb")
    nc.sync.dma_start(flag_bot[:], fb_d)
    w1 = {}
    for nm, d in w1_d.items():
        w1[nm] = sg.tile([C, C], BF16, name=nm + "_sb")
        nc.sync.dma_start(w1[nm][:], d)
    wd = {}
    for nm, d in wd_d.items():
        wd[nm] = sg.tile([C, 9, C], BF16, name=nm + "_sb")
        nc.sync.dma_start(wd[nm][:], d)
    bias = {}
    for nm, d in bias_d.items():
        bias[nm] = sg.tile([C, 1], F32, name=nm + "_sb")
        nc.sync.dma_start(bias[nm][:], d)

    onesC = sg.tile([C, C], BF16, name="onesC")
    nc.gpsimd.memset(onesC[:], 1.0 / C)
    ident = sg.tile([C, C], BF16, name="ident")
    make_identity(nc, ident[:])
    onesF = sg.tile([C, C], BF16, name="onesF")
    nc.gpsimd.memset(onesF[:], 1.0)
    zero_t = sg.tile([C, 1], F32, name="zero_t")
    nc.vector.memset(zero_t[:], 0.0)
    eps_t = sg.tile([C, 1], F32, name="eps_t")
    nc.vector.memset(eps_t[:], EPS)

    # ---- big activation tiles ----
    # K and V are computed as 32-row halves from the same halo as Q (the
    # three chains are structurally identical), then the full K/V are
    # assembled with a pair-wise AllGather (cores 2b and 2b+1 hold the two
    # halves of batch b).
    xnh_pad = sg.tile([C, 36, 66], BF16, name="xnh_pad")  # LN1(halo), padded
    k1p = sg.tile([C, 36, 66], BF16, name="k1p")
    v1p = sg.tile([C, 36, 66], BF16, name="v1p")
    q1p = sg.tile([C, 36, 66], BF16, name="q1p")
    s1p = sg.tile([C, 36, 66], BF16, name="s1p")
    KVh = sg.tile([C, HW], BF16, name="KVh")   # [K-half | V-half]
    K_cn = sg.tile([C, HW], BF16, name="K_cn")
    V_cn = sg.tile([C, HW], BF16, name="V_cn")
    Q_cn = sg.tile([C, NQ], BF16, name="Q_cn")
    VT_STRIDE = 160   # 320B row stride keeps each slice 32B-aligned for xbar DMA
    Vt_aug = sg.tile([C, 32, VT_STRIDE], BF16, name="Vt_aug")
    O_cn = sg.tile([C, NQ], BF16, name="O_cn")
    sum_sb = sg.tile([C, NQ], F32, name="sum_sb")
    out_sb = sg.tile([C, NQ], F32, name="out_sb")

    dramp = ctx.enter_context(tc.tile_pool(name="dramp", bufs=1, space="DRAM"))
    kv_in = dramp.tile([C, HW], BF16, name="kv_in")
    kv_out = dramp.tile([2 * C, HW], BF16, name="kv_out")

    # zero only the pad borders (full-tile memsets on the gpsimd queue were
    # delaying the LN pipeline); interiors are fully written by conv drains
    for t in (xnh_pad, k1p, v1p, q1p, s1p):
        rows = t.shape[1]
        nc.vector.memset(t[:, 0, :], 0.0)
        nc.vector.memset(t[:, rows - 1, :], 0.0)
        nc.gpsimd.memset(t[:, :, 0], 0.0)
        nc.gpsimd.memset(t[:, :, 65], 0.0)
    nc.vector.memset(Vt_aug[:, :, C:C + 1], 1.0)

    # ---- phase A pools ----
    phA = ExitStack()
    psLN = phA.enter_context(tc.tile_pool(name="psLN", bufs=2, space="PSUM"))
    psCV = phA.enter_context(tc.tile_pool(name="psCV", bufs=3, space="PSUM"))
    psW = phA.enter_context(tc.tile_pool(name="psW", bufs=1, space="PSUM"))
    lnp = phA.enter_context(tc.tile_pool(name="lnp", bufs=3))

    # PE warm-up: ~5us of back-to-back matmuls at t~0 trips the HAM clock
    # gate (cold PE runs at 1.2 GHz; the LN phase alone is too sparse to
    # warm it before the conv burst arrives).
    warm_ps = psW.tile([C, C], F32, name="warm_ps")
    for _ in range(24):
        nc.tensor.matmul(warm_ps[:], lhsT=onesC[:], rhs=onesC[:],
                         start=True, stop=True)

    def ln_chunk(src_ap, dst_ap, F):
        """LayerNorm-over-channels for one chunk of F tokens (no gamma/beta;
        gamma/beta folded into consumer conv weights host-side).
        src: (C,F) f32 SBUF ap; dst: (C,F)-shaped bf16 ap (possibly strided).
        rstd = 1/sqrt(var+eps): Sqrt on ACT (one table set), recip on DVE.
        mu^2 and var on GpSimd (frees DVE); mean-sub fused as (xb-mu)*R."""
        xb = lnp.tile([C, 512], BF16, name="ln_xb")
        nc.scalar.activation(xb[:, :F], src_ap, AF.Copy)
        sq = lnp.tile([C, 512], BF16, name="ln_sq")
        nc.gpsimd.tensor_mul(sq[:, :F], xb[:, :F], xb[:, :F])
        mu_p = psLN.tile([C, 512], F32, name="ln_mup")
        nc.tensor.matmul(mu_p[:, :F], lhsT=onesC[:], rhs=xb[:, :F],
                         start=True, stop=True)
        msq_p = psLN.tile([C, 512], F32, name="ln_msqp")
        nc.tensor.matmul(msq_p[:, :F], lhsT=onesC[:], rhs=sq[:, :F],
                         start=True, stop=True)
        mu = lnp.tile([C, 512], F32, name="ln_mu")
        nc.scalar.activation(mu[:, :F], mu_p[:, :F], AF.Copy)
        t2 = lnp.tile([C, 512], F32, name="ln_t2")
        nc.vector.tensor_mul(t2[:, :F], mu[:, :F], mu[:, :F])
        var = lnp.tile([C, 512], F32, name="ln_var")
        nc.vector.tensor_sub(var[:, :F], msq_p[:, :F], t2[:, :F])
        sd = lnp.tile([C, 512], F32, name="ln_sd")
        nc.scalar.activation(sd[:, :F], var[:, :F], AF.Sqrt, bias=eps_t[:])
        R = lnp.tile([C, 512], F32, name="ln_R")
        nc.vector.reciprocal_approx_fast(R[:, :F], sd[:, :F])
        xc = lnp.tile([C, 512], BF16, name="ln_xc")
        nc.vector.tensor_sub(xc[:, :F], xb[:, :F], mu_p[:, :F])
        nc.gpsimd.tensor_mul(dst_ap, xc[:, :F], R[:, :F])

    # (LN chunks are interleaved with the conv chains below to keep the
    #  TensorEngine fed -- a cold PE runs at 1.2 GHz until ~3.4us of
    #  sustained activity.)
    hgroups = [(0, 8), (8, 8), (16, 8), (24, 8), (32, 4)]

    def conv1x1(lhsT, rhs_ap, dst_ap, F, bias_ap, func=AF.Identity):
        ps = psCV.tile([C, 512], F32, name="cv_ps")
        nc.tensor.matmul(ps[:, :F], lhsT=lhsT, rhs=rhs_ap, start=True, stop=True)
        nc.scalar.activation(dst_ap, ps[:, :F], func, bias=bias_ap)

    def conv3x3(wtile, src_pad, row0, nr, dst_ap, bias_ap, func=AF.Identity):
        """out rows use src_pad rows row0+dy .. row0+dy+nr, cols dx..dx+64."""
        F = nr * W
        ps = psCV.tile([C, 512], F32, name="cv_ps")
        for s in range(9):
            dy, dx = s // 3, s % 3
            nc.tensor.matmul(
                ps[:, :F],
                lhsT=wtile[:, s, :],
                rhs=src_pad[:, row0 + dy:row0 + dy + nr, dx:dx + W],
                start=(s == 0), stop=(s == 8))
        nc.scalar.activation(dst_ap, ps[:, :F], func, bias=bias_ap)

    # ---- LN1 on the halo, interleaved with the q1/k1/v1 1x1 convs
    #      (group hi reads xnh rows 8hi+1..8hi+8 = LN chunks hi and hi+1,
    #       so the convs trail the LN chunks by one) ----
    qgroups = [(1, 8), (9, 8), (17, 8), (25, 8), (33, 2)]

    def c1group(hi):
        qi0, qnr = qgroups[hi]
        for wname, bname, dstp in (("wk1T", "bk1e", k1p), ("wv1T", "bv1e", v1p),
                                   ("wq1T", "bq1e", q1p)):
            conv1x1(w1[wname][:], xnh_pad[:, qi0:qi0 + qnr, 1:65],
                    dstp[:, qi0:qi0 + qnr, 1:65], qnr * W, bias[bname][:])

    def kvd_group(r):
        # K/V half deconv row-group r, streamed straight out to the
        # collective input buffer
        conv3x3(wd["wkdT"], k1p, 8 * r + 1, 8,
                KVh[:, 512 * r:512 * (r + 1)], bias["bkd"][:])
        nc.sync.dma_start(kv_in[:, 512 * r:512 * (r + 1)],
                          KVh[:, 512 * r:512 * (r + 1)])
        conv3x3(wd["wvdT"], v1p, 8 * r + 1, 8,
                KVh[:, NQ + 512 * r:NQ + 512 * (r + 1)], bias["bvd"][:])
        nc.sync.dma_start(kv_in[:, NQ + 512 * r:NQ + 512 * (r + 1)],
                          KVh[:, NQ + 512 * r:NQ + 512 * (r + 1)])

    # LN chunks -> 1x1 convs (trailing 1) -> K/V deconv groups (trailing 2),
    # so the AllGather can issue as early as possible.
    for hi, (i0, nr) in enumerate(hgroups):
        F = nr * W
        ln_chunk(xh[:, i0 * W:i0 * W + F],
                 xnh_pad[:, i0:i0 + nr, 1:65], F)
        if hi >= 1:
            c1group(hi - 1)
        if hi == 2:
            for t in (k1p, v1p, q1p):
                nc.vector.tensor_scalar_mul(t[:, 1, 1:65], t[:, 1, 1:65],
                                            flag_top[:])
        if hi >= 2:
            kvd_group(hi - 2)
    c1group(4)
    for t in (k1p, v1p, q1p):
        nc.vector.tensor_scalar_mul(t[:, 34, 1:65], t[:, 34, 1:65], flag_bot[:])
    kvd_group(3)

    nc.gpsimd.collective_compute(
        "AllGather", mybir.AluOpType.bypass,
        ins=[kv_in[:]], outs=[kv_out[:]],
        replica_groups=[[0, 1], [2, 3], [4, 5], [6, 7]])

    # ---- work that overlaps the collective: Q deconv, sc chain ----
    for r in range(4):
        conv3x3(wd["wqdT"], q1p, 8 * r + 1, 8,
                Q_cn[:, 512 * r:512 * (r + 1)], bias["bqd"][:])

    sgroups = [(0, 8), (8, 8), (16, 8), (24, 8), (32, 2)]   # 34 s1 rows
    for (i0, nr) in sgroups:   # s1 rows i0..i0+nr-1 (abs r0-1+i0..)
        conv3x3(wd["ws1T"], xnh_pad, i0, nr,
                s1p[:, 1 + i0:1 + i0 + nr, 1:65], bias["bs1e"][:], func=AF.Relu)
    nc.vector.tensor_scalar_mul(s1p[:, 1, 1:65], s1p[:, 1, 1:65], flag_top[:])
    nc.vector.tensor_scalar_mul(s1p[:, 34, 1:65], s1p[:, 34, 1:65], flag_bot[:])
    for r in range(4):
        conv3x3(wd["ws2T"], s1p, 8 * r + 1, 8,
                sum_sb[:, 512 * r:512 * (r + 1)], bias["bs2bo"][:])
    # add residual (raw x rows r0..r0+31 live in xh rows 2..33)
    for r in range(4):
        nc.vector.tensor_add(sum_sb[:, 512 * r:512 * (r + 1)],
                             sum_sb[:, 512 * r:512 * (r + 1)],
                             xh[:, 2 * W + 512 * r:2 * W + 512 * (r + 1)])

    # ---- assemble full K/V from the gathered halves; V transposed on the
    #      TensorEngine (the xbar DMA path serialized ~39us on one queue) ----
    for half in range(2):
        nc.sync.dma_start(K_cn[:, NQ * half:NQ * (half + 1)],
                          kv_out[C * half:C * (half + 1), 0:NQ])
        nc.sync.dma_start(V_cn[:, NQ * half:NQ * (half + 1)],
                          kv_out[C * half:C * (half + 1), NQ:HW])

    phA.close()

    def ln2_chunk(src_ap, dst_ap, F):
        # final LayerNorm chunk: f32 apply with ln2 gamma/beta
        xb = lnp2.tile([C, 512], BF16, name="l2_xb")
        nc.vector.tensor_copy(xb[:, :F], src_ap)
        sq = lnp2.tile([C, 512], BF16, name="l2_sq")
        nc.vector.tensor_mul(sq[:, :F], xb[:, :F], xb[:, :F])
        mu_p = psL2.tile([C, 512], F32, name="l2_mup")
        nc.tensor.matmul(mu_p[:, :F], lhsT=onesC[:], rhs=xb[:, :F],
                         start=True, stop=True)
        msq_p = psL2.tile([C, 512], F32, name="l2_msqp")
        nc.tensor.matmul(msq_p[:, :F], lhsT=onesC[:], rhs=sq[:, :F],
                         start=True, stop=True)
        mu = lnp2.tile([C, 512], F32, name="l2_mu")
        nc.vector.tensor_copy(mu[:, :F], mu_p[:, :F])
        t2 = lnp2.tile([C, 512], F32, name="l2_t2")
        nc.vector.tensor_mul(t2[:, :F], mu[:, :F], mu[:, :F])
        var = lnp2.tile([C, 512], F32, name="l2_var")
        nc.vector.tensor_sub(var[:, :F], msq_p[:, :F], t2[:, :F])
        sd = lnp2.tile([C, 512], F32, name="l2_sd")
        nc.scalar.activation(sd[:, :F], var[:, :F], AF.Sqrt, bias=eps_t[:])
        R = lnp2.tile([C, 512], F32, name="l2_R")
        nc.vector.reciprocal_approx_fast(R[:, :F], sd[:, :F])
        xc = lnp2.tile([C, 512], F32, name="l2_xc")
        nc.vector.tensor_sub(xc[:, :F], src_ap, mu_p[:, :F])
        xcr = lnp2.tile([C, 512], F32, name="l2_xcr")
        nc.vector.tensor_mul(xcr[:, :F], xc[:, :F], R[:, :F])
        nc.vector.tensor_scalar(
            out=dst_ap, in0=xcr[:, :F],
            scalar1=bias["ln2_g"][:], scalar2=bias["ln2_b"][:],
            op0=mybir.AluOpType.mult, op1=mybir.AluOpType.add)

    # ---- attention ----
    phB = ExitStack()
    psS = phB.enter_context(tc.tile_pool(name="psS", bufs=2, space="PSUM"))
    psO = phB.enter_context(tc.tile_pool(name="psO", bufs=2, space="PSUM"))
    psC = phB.enter_context(tc.tile_pool(name="psC", bufs=1, space="PSUM"))
    psT = phB.enter_context(tc.tile_pool(name="psT", bufs=2, space="PSUM"))
    sbE = phB.enter_context(tc.tile_pool(name="sbE", bufs=4))
    sbOT = phB.enter_context(tc.tile_pool(name="sbOT", bufs=2))

    for m in range(32):
        tp = psT.tile([C, C], BF16, name="vt_ps")
        nc.tensor.transpose(tp[:], V_cn[:, C * m:C * (m + 1)], ident[:])
        nc.vector.tensor_copy(Vt_aug[:, m, 0:C], tp[:])

    for g in range(4):
        qs = Q_cn[:, 512 * g:512 * (g + 1)]
        o_ps = psO.tile([C, 512], F32, name="o_ps")
        den_ps = psO.tile([C, 512], F32, name="den_ps", bufs=1)
        for m in range(32):
            s_ps = psS.tile([C, 512], F32, name="s_ps")
            nc.tensor.matmul(s_ps[:], lhsT=K_cn[:, C * m:C * (m + 1)],
                             rhs=qs, start=True, stop=True)
            e = sbE.tile([C, 512], BF16, name="e_t")
            nc.scalar.activation(e[:], s_ps[:], AF.Exp, scale=SCALE, bias=zero_t[:])
            nc.tensor.matmul(o_ps[:], lhsT=Vt_aug[:, m, 0:C], rhs=e[:],
                             start=(m == 0), stop=(m == 31))
            nc.tensor.matmul(den_ps[:], lhsT=onesF[:], rhs=e[:],
                             start=(m == 0), stop=(m == 31))
        ro = sbOT.tile([C, 512], BF16, name="ro")
        nc.scalar.activation(ro[:], o_ps[:], AF.Relu, bias=zero_t[:])
        rden = sbOT.tile([C, 512], F32, name="rden")
        nc.vector.reciprocal_approx_fast(rden[:], den_ps[:])
        nc.vector.tensor_mul(O_cn[:, 512 * g:512 * (g + 1)], ro[:], rden[:])
        o2 = psC.tile([C, 512], F32, name="wo_ps")
        nc.tensor.matmul(o2[:], lhsT=w1["woT"][:],
                         rhs=O_cn[:, 512 * g:512 * (g + 1)], start=True, stop=True)
        nc.vector.tensor_add(sum_sb[:, 512 * g:512 * (g + 1)],
                             sum_sb[:, 512 * g:512 * (g + 1)], o2[:])
    phB.close()

    phC = ExitStack()
    psL2 = phC.enter_context(tc.tile_pool(name="psL2", bufs=2, space="PSUM"))
    lnp2 = phC.enter_context(tc.tile_pool(name="lnp2", bufs=2))
    for g in range(4):
        ln2_chunk(sum_sb[:, 512 * g:512 * (g + 1)],
                  out_sb[:, 512 * g:512 * (g + 1)], 512)
    phC.close()

    for g in range(4):
        nc.sync.dma_start(out_d[:, 512 * g:512 * (g + 1)],
                          out_sb[:, 512 * g:512 * (g + 1)])


# --------------------------------------------------------------------------
# public entry point
# --------------------------------------------------------------------------

_CACHED = {}


def _get_nc():
    if "nc" not in _CACHED:
        _CACHED["nc"] = _build()
    return _CACHED["nc"]


def kernel(x, params):
    x = np.asarray(x, np.float32)
    wp = _prep_weights(params)
    in_maps = _prep_core_inputs(x, wp)
    nc = _get_nc()
    res = run_bass_kernel_spmd(nc, in_maps, list(range(NCORES)))
    out = np.empty((4, C, H, W), np.float32)
    for core in range(NCORES):
        b, h = core // 2, core % 2
        r0 = 32 * h
        out[b, :, r0:r0 + 32, :] = res.results[core]["out"].reshape(C, 32, W)
    return out
